# revision 1
# baseline (speedup 1.0000x reference)
"""Trainium2 Bass kernel for single-head attention (B=8, S=2048, E=768).

Data-parallel over batch: core c computes batch c entirely.

Host-side packing (weight fusion + layout marshalling):
  Wkq  = Wk.T @ Wq           (bf16)  -- q/k projections fused into scores
  WvoT = (Wo @ Wv).T         (fp32r) -- v/out projections fused
  query transposed+cast bf16 -> queryT [E,S]; key gathered to the unmasked
  set (padded with masked keys, which contribute exp(-200)=0 exactly),
  transposed+cast bf16 -> keyT [E,nkeys]; value gathered, relabeled fp32r.

Per-core device dataflow (PE contraction dim = partition dim):
  Hk[e',j] = sum_e Wkq[e,e'] xkT[e,j]         (bf16)
  sT[j,i]  = sum_e' Hk[e',j] xqT[e',i]        = raw q.k scores
  aT[j,i]  = exp(sT/768 + maskbias[j])        (ACT)
  den[i]   = sum_j aT[j,i]                    (ones-matmul, replicated)
  U[e,i]   = sum_j xv[j,e] aT[j,i]            (xv natural layout)
  Un[e,i]  = U[e,i] / den[i]
  y[i,o]   = sum_e Un[e,i] WvoT[e,o] + bo[o]
"""

import numpy as np

S, E, P = 2048, 768, 128
NE, NS = E // P, S // P  # 6, 16
IC = 512                 # attention i-chunk
NIC = S // IC            # 4
N_CORES = 8
NKC = 1152               # compacted key count (9 j-tiles); P(>NKC) ~ 1e-8

_CACHE = {}


def _chunks(total, step=512):
    out = []
    o = 0
    while o < total:
        out.append((o, min(step, total - o)))
        o += step
    return out


def build_nc(n_iters=1, nkeys=NKC):
    from contextlib import ExitStack

    import concourse.bacc as bacc
    import concourse.bass as bass
    import concourse.mybir as mybir
    import concourse.tile as tile

    F32 = mybir.dt.float32
    F32R = mybir.dt.float32r
    BF16 = mybir.dt.bfloat16
    I32 = mybir.dt.int32
    AF = mybir.ActivationFunctionType
    ALU = mybir.AluOpType

    KJ = nkeys // P
    nc = bacc.Bacc("TRN2", target_bir_lowering=False, debug=False,
                   num_devices=N_CORES)

    xq_d = nc.dram_tensor("queryT", [E, S], BF16, kind="ExternalInput").ap()
    xk_d = nc.dram_tensor("keyT", [E, nkeys], BF16, kind="ExternalInput").ap()
    xv_d = nc.dram_tensor("value", [nkeys, E], F32R, kind="ExternalInput").ap()
    mask_d = nc.dram_tensor("mask", [nkeys], I32, kind="ExternalInput").ap()
    wkq_d = nc.dram_tensor("Wkq", [E, E], BF16, kind="ExternalInput").ap()
    wvot_d = nc.dram_tensor("WvoT", [E, E], F32R, kind="ExternalInput").ap()
    bo_d = nc.dram_tensor("bo", [E], F32, kind="ExternalInput").ap()
    y_d = nc.dram_tensor("out", [S, E], F32, kind="ExternalOutput").ap()

    with tile.TileContext(nc) as tc:
      for _it in range(n_iters):
       with ExitStack() as ctx:
        persist = ctx.enter_context(tc.tile_pool(name="persist", bufs=1))

        xqT = persist.tile([P, NE, S], BF16)
        hk = persist.tile([P, NE, nkeys], BF16)
        xv_r = persist.tile([P, KJ, E], F32R)
        wvoT = persist.tile([P, NE, E], F32R)
        maskb = persist.tile([P, KJ], F32)
        ones_r = persist.tile([P, P], F32R)
        bo_rep = persist.tile([P, E], F32)

        ones_f = persist.tile([P, P], F32)
        nc.vector.memset(ones_f, 1.0)
        nc.vector.tensor_copy(out=ones_r, in_=ones_f)

        bo_bc = bass.AP(tensor=bo_d.tensor, offset=bo_d.offset,
                        ap=[[0, P]] + list(bo_d.ap))
        nc.sync.dma_start(out=bo_rep, in_=bo_bc)

        mask_sb = persist.tile([P, KJ], I32)
        nc.sync.dma_start(out=mask_sb, in_=mask_d.rearrange("(t p) -> p t", p=P))
        mask_f = persist.tile([P, KJ], F32)
        nc.vector.tensor_copy(out=mask_f, in_=mask_sb)
        nc.vector.tensor_scalar(out=maskb, in0=mask_f, scalar1=200.0,
                                scalar2=-200.0, op0=ALU.mult, op1=ALU.add)

        nc.sync.dma_start(out=wvoT,
                          in_=wvot_d.rearrange("(t p) o -> p t o", p=P))
        nc.sync.dma_start(out=xv_r,
                          in_=xv_d.rearrange("(t p) e -> p t e", p=P))

        # ------------- phase 1: loads + Hk = WqkT.T @ xkT -------------
        with tc.tile_pool(name="wt", bufs=1) as wt_pool, \
             tc.tile_pool(name="xkt", bufs=1) as xkt_pool, \
             tc.tile_pool(name="pp", bufs=1, space="PSUM") as psum_p:

            nc.sync.dma_start(
                out=xqT, in_=xq_d.rearrange("(t p) i -> p t i", p=P))
            xkT = xkt_pool.tile([P, NE, nkeys], BF16, tag="xkt")
            nc.sync.dma_start(
                out=xkT, in_=xk_d.rearrange("(t p) j -> p t j", p=P))

            wkq = wt_pool.tile([P, NE, E], BF16, tag="wt")
            nc.sync.dma_start(
                out=wkq, in_=wkq_d.rearrange("(t p) o -> p t o", p=P))

            for ept in range(NE):   # e' tile of Hk rows
                ps = psum_p.tile([P, S], F32, tag="pp", name=f"hk{_it}_{ept}")
                for o0, on in _chunks(nkeys):
                    for et in range(NE):
                        nc.tensor.matmul(
                            ps[:, o0:o0 + on],
                            lhsT=wkq[:, et, ept * P:(ept + 1) * P],
                            rhs=xkT[:, et, o0:o0 + on],
                            start=(et == 0), stop=(et == NE - 1))
                nc.vector.tensor_copy(out=hk[:, ept, :], in_=ps[:, :nkeys])

        # ---------------- phase 2: attention + output ----------------
        with tc.tile_pool(name="at", bufs=3) as attn_pool, \
             tc.tile_pool(name="un", bufs=2) as un_pool, \
             tc.tile_pool(name="rc", bufs=2) as recip_pool, \
             tc.tile_pool(name="ys", bufs=3) as y_pool, \
             tc.tile_pool(name="zp", bufs=1, space="PSUM") as psum_z, \
             tc.tile_pool(name="dp", bufs=1, space="PSUM") as psum_d, \
             tc.tile_pool(name="sp", bufs=1, space="PSUM") as psum_s:

            for ic in range(NIC):
                isl = slice(ic * IC, (ic + 1) * IC)
                u_big = psum_z.tile([P, NE * IC], F32, tag="z",
                                    name=f"ub{_it}_{ic}")
                den_ps = psum_d.tile([P, IC], F32, tag="d",
                                     name=f"dp{_it}_{ic}")
                for jt in range(KJ):
                    s_ps = psum_s.tile([P, IC], F32, tag="s",
                                       name=f"sp{_it}_{ic}_{jt}")
                    for ept in range(NE):
                        nc.tensor.matmul(
                            s_ps,
                            lhsT=hk[:, ept, jt * P:(jt + 1) * P],
                            rhs=xqT[:, ept, isl],
                            start=(ept == 0), stop=(ept == NE - 1))
                    at = attn_pool.tile([P, IC], F32R, tag="at")
                    nc.scalar.activation(
                        out=at, in_=s_ps, func=AF.Exp,
                        bias=maskb[:, jt:jt + 1], scale=1.0 / float(E))
                    nc.tensor.matmul(
                        den_ps, lhsT=ones_r, rhs=at,
                        start=(jt == 0), stop=(jt == KJ - 1))
                    for et in range(NE):
                        nc.tensor.matmul(
                            u_big[:, et * IC:(et + 1) * IC],
                            lhsT=xv_r[:, jt, et * P:(et + 1) * P],
                            rhs=at,
                            start=(jt == 0), stop=(jt == KJ - 1))
                recip = recip_pool.tile([P, IC], F32, tag="rc")
                nc.vector.reciprocal(recip, den_ps)
                unorm = un_pool.tile([P, NE, IC], F32R, tag="un")
                recip_bc = bass.AP(tensor=recip.tensor, offset=recip.offset,
                                   ap=[recip.ap[0], [0, NE], recip.ap[1]])
                nc.vector.tensor_tensor(
                    out=unorm, in0=u_big.rearrange("p (n i) -> p n i", n=NE),
                    in1=recip_bc, op=ALU.mult)
                for it in range(IC // P):
                    y_big = psum_z.tile([P, E], F32, tag="z",
                                        name=f"yb{_it}_{ic}_{it}")
                    for o0, on in ((0, 512), (512, 256)):
                        for et in range(NE):
                            nc.tensor.matmul(
                                y_big[:, o0:o0 + on],
                                lhsT=unorm[:, et, it * P:(it + 1) * P],
                                rhs=wvoT[:, et, o0:o0 + on],
                                start=(et == 0), stop=(et == NE - 1))
                    ysb = y_pool.tile([P, E], F32, tag="y")
                    nc.vector.tensor_tensor(out=ysb, in0=y_big, in1=bo_rep,
                                            op=ALU.add)
                    r0 = ic * IC + it * P
                    nc.sync.dma_start(out=y_d[r0:r0 + P, :], in_=ysb)

    nc.compile()
    return nc


def get_nc(n_iters=1, nkeys=NKC):
    key = ("nc", n_iters, nkeys)
    if key not in _CACHE:
        _CACHE[key] = build_nc(n_iters, nkeys)
    return _CACHE[key]


def pack_inputs(value, key, query, mask, Wv, Wk, Wq, Wo, bo):
    """Host-side packing: per-core input maps (weight fusion + layouts)."""
    import ml_dtypes

    value = np.asarray(value, dtype=np.float32)
    key = np.asarray(key, dtype=np.float32)
    query = np.asarray(query, dtype=np.float32)
    mask = np.asarray(mask, dtype=np.int32)
    Wv = np.asarray(Wv, dtype=np.float32)
    Wk = np.asarray(Wk, dtype=np.float32)
    Wq = np.asarray(Wq, dtype=np.float32)
    Wo = np.asarray(Wo, dtype=np.float32)
    bo = np.asarray(bo, dtype=np.float32)

    wkq = np.ascontiguousarray(Wk.T @ Wq).astype(ml_dtypes.bfloat16)
    wvo = Wo @ Wv
    wvot = np.ascontiguousarray(wvo.T)

    # key compaction: keep unmasked keys, pad with masked ones (exp -> 0)
    idxs = []
    nkeys = NKC
    for c in range(N_CORES):
        m = mask[c, 0]
        keep = np.flatnonzero(m != 0)
        drop = np.flatnonzero(m == 0)
        if len(keep) > NKC or len(drop) == 0:
            nkeys = S
            break
        pad = np.full(NKC - len(keep), drop[0], dtype=np.int64)
        idxs.append(np.concatenate([keep, pad]))

    in_maps = []
    for c in range(N_CORES):
        if nkeys == S:
            kc, vc, mc = key[c], value[c], mask[c, 0]
        else:
            ix = idxs[c]
            kc, vc, mc = key[c][ix], value[c][ix], mask[c, 0][ix]
        in_maps.append({
            "queryT": np.ascontiguousarray(
                query[c].T).astype(ml_dtypes.bfloat16),
            "keyT": np.ascontiguousarray(kc.T).astype(ml_dtypes.bfloat16),
            "value": np.ascontiguousarray(vc),
            "mask": np.ascontiguousarray(mc),
            "Wkq": wkq, "WvoT": wvot,
            "bo": bo,
        })
    return in_maps, nkeys


def kernel(**inputs):
    from concourse.bass_utils import run_bass_kernel_spmd

    in_maps, nkeys = pack_inputs(
        inputs["value"], inputs["key"], inputs["query"], inputs["mask"],
        inputs["Wv"], inputs["Wk"], inputs["Wq"], inputs["Wo"], inputs["bo"])
    nc = get_nc(nkeys=nkeys)
    res = run_bass_kernel_spmd(nc, in_maps, list(range(N_CORES)))
    out = np.stack([res.results[c]["out"] for c in range(N_CORES)], axis=0)
    return out



# revision 6
# speedup vs baseline: 72.5574x; 72.5574x over previous
"""Trainium2 Bass kernel for single-head attention (B=8, S=2048, E=768).

Data-parallel over batch: core c computes batch c entirely.

Host-side packing (weight fusion + layout marshalling):
  Wkq   = Wk.T @ Wq / E      (bf16)  -- q/k projections fused into scores,
                                        with the 1/E softmax scale folded in
  WvoT2 = (Wo @ Wv).T        (bf16)  -- v/out projections fused, applied to
                                        value BEFORE attention (nkeys < S)
  query/key/value transposed+cast bf16 -> [E, *]; key/value gathered to the
  unmasked set (padded with masked keys, which contribute exp(-200)=0).
  maskb = 0 / -200 bias per key (fp32), precomputed on host.

Per-core device dataflow (PE contraction dim = partition dim):
  Hk[e',j]  = sum_e Wkq[e,e'] keyT[e,j]        (bf16)
  xv'[j,o]  = sum_e valueT[e,j] WvoT2[e,o] + bo[o]   (bf16; col 768 = 1.0)
  sT[j,i]   = sum_e' Hk[e',j] queryT[e',i]     = scaled q.k scores
  aT[j,i]   = exp(sT + maskb[j])               (ACT, bf16)
  U[i,o+]   = sum_j aT[j,i] xv'[j,o+]          (o+ includes ones col -> den[i])
  y[i,o]    = U[i,o] / U[i,768]                (recip + broadcast mult)
Since sum_j a[j,i]*(xv+bo)[j,o] = U[i,o] + den[i]*bo[o], normalizing by den
adds bo exactly. Output leaves in natural [S, E] orientation.
"""

import numpy as np

S, E, P = 2048, 768, 128
NE = E // P              # 6
IC = 512                 # score i-chunk
NIC = S // IC            # 4
N_CORES = 8
NKC = 1152               # compacted key count (9 j-tiles)
EP1 = E + 1              # 769: value' cols + ones column
EPAD = 772               # padded row length for xv' tile

_CACHE = {}


def _chunks(total, step=512):
    out = []
    o = 0
    while o < total:
        out.append((o, min(step, total - o)))
        o += step
    return out


def build_nc(n_iters=1, nkeys=NKC):
    import concourse.bacc as bacc
    import concourse.bass as bass
    import concourse.mybir as mybir
    import concourse.tile as tile

    F32 = mybir.dt.float32
    BF16 = mybir.dt.bfloat16
    F8 = mybir.dt.float8e4
    DR = mybir.MatmulPerfMode.DoubleRow
    AF = mybir.ActivationFunctionType
    ALU = mybir.AluOpType

    KJ = nkeys // P
    nc = bacc.Bacc("TRN2", target_bir_lowering=False, debug=False,
                   num_devices=N_CORES)

    xq_d = nc.dram_tensor("queryT", [E, S], F8, kind="ExternalInput").ap()
    xk_d = nc.dram_tensor("keyT", [E, nkeys], F8, kind="ExternalInput").ap()
    xv_d = nc.dram_tensor("valueT", [E, nkeys], BF16, kind="ExternalInput").ap()
    mb_d = nc.dram_tensor("maskb", [nkeys], F32, kind="ExternalInput").ap()
    wkq_d = nc.dram_tensor("Wkq", [E, E], F8, kind="ExternalInput").ap()
    wvot_d = nc.dram_tensor("WvoT2", [E, E], BF16, kind="ExternalInput").ap()
    bo_d = nc.dram_tensor("bo", [E], F32, kind="ExternalInput").ap()
    y_d = nc.dram_tensor("out", [S, E], F32, kind="ExternalOutput").ap()

    with tile.TileContext(nc) as tc, \
         tc.tile_pool(name="persist", bufs=1) as persist, \
         tc.tile_pool(name="ld2", bufs=2) as ld2, \
         tc.tile_pool(name="ld1", bufs=1) as ld1, \
         tc.tile_pool(name="rc", bufs=2) as recip_pool, \
         tc.tile_pool(name="ys", bufs=3) as y_pool, \
         tc.tile_pool(name="sp", bufs=2, space="PSUM") as psum_s, \
         tc.tile_pool(name="up", bufs=2, space="PSUM") as psum_u:

      for _it in range(n_iters):
        hk = persist.tile([P, NE, nkeys], F8, tag="hk")
        xvp = persist.tile([P, KJ, EPAD], BF16, tag="xvp")
        atile = persist.tile([P, KJ, IC], BF16, tag="at")

        # loads, in dependency order: phase A1 deps first, xqT last
        xkT = ld2.tile([P, NE, nkeys], F8, tag="xkt")
        nc.sync.dma_start(out=xkT, in_=xk_d.rearrange("(t p) j -> p t j", p=P))
        wkq = ld1.tile([P, NE, E], F8, tag="wt")
        nc.sync.dma_start(out=wkq, in_=wkq_d.rearrange("(t p) o -> p t o", p=P))
        xvT = ld2.tile([P, NE, nkeys], BF16, tag="xvt")
        nc.sync.dma_start(out=xvT, in_=xv_d.rearrange("(t p) j -> p t j", p=P))
        wvot = ld1.tile([P, NE, E], BF16, tag="wv")
        nc.sync.dma_start(out=wvot, in_=wvot_d.rearrange("(t p) o -> p t o", p=P))
        maskb = ld1.tile([P, KJ], F32, tag="mb")
        nc.sync.dma_start(out=maskb, in_=mb_d.rearrange("(t p) -> p t", p=P))
        bo_rep = ld1.tile([P, E], F32, tag="bo")
        bo_bc = bass.AP(tensor=bo_d.tensor, offset=bo_d.offset,
                        ap=[[0, P]] + list(bo_d.ap))
        nc.sync.dma_start(out=bo_rep, in_=bo_bc)
        xqT = ld2.tile([P, NE, S], F8, tag="xqt")
        nc.sync.dma_start(out=xqT, in_=xq_d.rearrange("(t p) i -> p t i", p=P))

        nc.vector.memset(xvp[:, :, E:E + 1], 1.0)

        # ------------- phase A1: Hk = Wkq.T @ keyT -------------
        for ept in range(NE):   # e' tile of Hk rows
            for o0, on in _chunks(nkeys):
                ps = psum_s.tile([P, on], F32, tag="s",
                                 name=f"hk{_it}_{ept}_{o0}")
                for et in range(NE // 2):
                    nc.tensor.matmul(
                        ps,
                        lhsT=wkq[:, 2 * et:2 * et + 2, ept * P:(ept + 1) * P],
                        rhs=xkT[:, 2 * et:2 * et + 2, o0:o0 + on],
                        start=(et == 0), stop=(et == NE // 2 - 1),
                        perf_mode=DR)
                nc.vector.tensor_copy(out=hk[:, ept, o0:o0 + on], in_=ps)

        # ------------- phase A2: xv' = valueT.T @ WvoT2 + bo -------------
        for jt in range(KJ):
            ps = psum_u.tile([P, E], F32, tag="u", name=f"xv{_it}_{jt}")
            for o0, on in _chunks(E):
                for et in range(NE):
                    nc.tensor.matmul(
                        ps[:, o0:o0 + on],
                        lhsT=xvT[:, et, jt * P:(jt + 1) * P],
                        rhs=wvot[:, et, o0:o0 + on],
                        start=(et == 0), stop=(et == NE - 1))
            nc.vector.tensor_tensor(
                out=xvp[:, jt, 0:E], in0=ps, in1=bo_rep, op=ALU.add)

        # ---------------- phase B: attention + output ----------------
        for ic in range(NIC):
            isl = slice(ic * IC, (ic + 1) * IC)
            for jt in range(KJ):
                s_ps = psum_s.tile([P, IC], F32, tag="s",
                                   name=f"sp{_it}_{ic}_{jt}")
                for ept in range(NE // 2):
                    nc.tensor.matmul(
                        s_ps,
                        lhsT=hk[:, 2 * ept:2 * ept + 2, jt * P:(jt + 1) * P],
                        rhs=xqT[:, 2 * ept:2 * ept + 2, isl],
                        start=(ept == 0), stop=(ept == NE // 2 - 1),
                        perf_mode=DR)
                nc.scalar.activation(
                    out=atile[:, jt, :], in_=s_ps, func=AF.Exp,
                    bias=maskb[:, jt:jt + 1], scale=1.0 / 1024.0)
            for it in range(IC // P):
                u_ps = psum_u.tile([P, EPAD], F32, tag="u",
                                   name=f"up{_it}_{ic}_{it}")
                for o0, on in ((0, 512), (512, EP1 - 512)):
                    for jt in range(KJ):
                        nc.tensor.matmul(
                            u_ps[:, o0:o0 + on],
                            lhsT=atile[:, jt, it * P:(it + 1) * P],
                            rhs=xvp[:, jt, o0:o0 + on],
                            start=(jt == 0), stop=(jt == KJ - 1))
                recip = recip_pool.tile([P, 1], F32, tag="rc")
                nc.vector.reciprocal(recip, u_ps[:, E:EP1])
                recip_bc = bass.AP(tensor=recip.tensor, offset=recip.offset,
                                   ap=[recip.ap[0], [0, E]])
                ysb = y_pool.tile([P, E], F32, tag="y")
                nc.vector.tensor_tensor(
                    out=ysb, in0=u_ps[:, 0:E], in1=recip_bc, op=ALU.mult)
                r0 = ic * IC + it * P
                nc.sync.dma_start(out=y_d[r0:r0 + P, :], in_=ysb)

    nc.compile()
    return nc


def get_nc(n_iters=1, nkeys=NKC):
    key = ("nc", n_iters, nkeys)
    if key not in _CACHE:
        _CACHE[key] = build_nc(n_iters, nkeys)
    return _CACHE[key]


def pack_inputs(value, key, query, mask, Wv, Wk, Wq, Wo, bo):
    """Host-side packing: per-core input maps (weight fusion + layouts)."""
    import ml_dtypes

    value = np.asarray(value, dtype=np.float32)
    key = np.asarray(key, dtype=np.float32)
    query = np.asarray(query, dtype=np.float32)
    mask = np.asarray(mask, dtype=np.int32)
    Wv = np.asarray(Wv, dtype=np.float32)
    Wk = np.asarray(Wk, dtype=np.float32)
    Wq = np.asarray(Wq, dtype=np.float32)
    Wo = np.asarray(Wo, dtype=np.float32)
    bo = np.asarray(bo, dtype=np.float32)

    wkq = np.ascontiguousarray(
        Wk.T @ Wq * (1024.0 / float(E))).astype(ml_dtypes.float8_e4m3)
    wvot = np.ascontiguousarray((Wo @ Wv).T).astype(ml_dtypes.bfloat16)

    # key compaction: keep unmasked keys, pad with masked ones (exp -> 0)
    idxs = []
    nkeys = NKC
    for c in range(N_CORES):
        m = mask[c, 0]
        keep = np.flatnonzero(m != 0)
        drop = np.flatnonzero(m == 0)
        if len(keep) > NKC or len(drop) == 0:
            nkeys = S
            break
        pad = np.full(NKC - len(keep), drop[0], dtype=np.int64)
        idxs.append(np.concatenate([keep, pad]))

    in_maps = []
    for c in range(N_CORES):
        if nkeys == S:
            kc, vc, mc = key[c], value[c], mask[c, 0]
        else:
            ix = idxs[c]
            kc, vc, mc = key[c][ix], value[c][ix], mask[c, 0][ix]
        maskb = np.where(mc != 0, 0.0, -200.0).astype(np.float32)
        in_maps.append({
            "queryT": np.ascontiguousarray(
                query[c].T).astype(ml_dtypes.float8_e4m3),
            "keyT": np.ascontiguousarray(kc.T).astype(ml_dtypes.float8_e4m3),
            "valueT": np.ascontiguousarray(vc.T).astype(ml_dtypes.bfloat16),
            "maskb": np.ascontiguousarray(maskb),
            "Wkq": wkq, "WvoT2": wvot,
            "bo": bo,
        })
    return in_maps, nkeys


def _make_runner(nc, n_cores):
    """Build a CACHED jitted executable for `nc` (sharded over n_cores).

    run_bass_kernel_spmd re-jits a fresh closure per call, so every call
    re-traces + re-serializes the NEFF. Building the jit once and reusing
    it makes repeat calls pay only dispatch + transfers + execution.
    """
    import jax
    from jax.sharding import Mesh, PartitionSpec
    from jax.experimental.shard_map import shard_map

    import concourse.mybir as mybir
    from concourse.bass2jax import (
        _bass_exec_p, install_neuronx_cc_hook, partition_id_tensor)

    install_neuronx_cc_hook()
    partition_name = (nc.partition_id_tensor.name
                      if nc.partition_id_tensor else None)
    in_names, out_names, out_avals, zero_outs = [], [], [], []
    for alloc in nc.m.functions[0].allocations:
        if not isinstance(alloc, mybir.MemoryLocationSet):
            continue
        name = alloc.memorylocations[0].name
        if alloc.kind == "ExternalInput":
            if name != partition_name:
                in_names.append(name)
        elif alloc.kind == "ExternalOutput":
            out_names.append(name)
            shape = tuple(alloc.tensor_shape)
            dtype = mybir.dt.np(alloc.dtype)
            out_avals.append(jax.core.ShapedArray(shape, dtype))
            zero_outs.append(np.zeros(shape, dtype))
    n_params = len(in_names)
    all_in_names = list(in_names) + list(out_names)
    if partition_name is not None:
        all_in_names.append(partition_name)

    def _body(*args):
        operands = list(args)
        if partition_name is not None:
            operands.append(partition_id_tensor())
        outs = _bass_exec_p.bind(
            *operands,
            out_avals=tuple(out_avals),
            in_names=tuple(all_in_names),
            out_names=tuple(out_names),
            lowering_input_output_aliases=(),
            sim_require_finite=True,
            sim_require_nnan=True,
            nc=nc,
        )
        return tuple(outs)

    devices = jax.devices()[:n_cores]
    mesh = Mesh(np.asarray(devices), ("core",))
    in_specs = (PartitionSpec("core"),) * (n_params + len(out_names))
    out_specs = (PartitionSpec("core"),) * len(out_names)
    fn = jax.jit(shard_map(_body, mesh=mesh, in_specs=in_specs,
                           out_specs=out_specs, check_rep=False))
    concat_zeros = [np.zeros((n_cores * z.shape[0], *z.shape[1:]), z.dtype)
                    for z in zero_outs]

    def run(in_maps):
        per_core = [[np.asarray(m[name]) for name in in_names]
                    for m in in_maps]
        concat_in = [
            np.concatenate([per_core[c][i] for c in range(n_cores)], axis=0)
            for i in range(n_params)]
        out_arrs = fn(*concat_in, *concat_zeros)
        return [
            {name: np.asarray(out_arrs[i]).reshape(
                n_cores, *out_avals[i].shape)[c]
             for i, name in enumerate(out_names)}
            for c in range(n_cores)]

    return run


def get_runner(n_iters=1, nkeys=NKC):
    key = ("runner", n_iters, nkeys)
    if key not in _CACHE:
        _CACHE[key] = _make_runner(get_nc(n_iters, nkeys), N_CORES)
    return _CACHE[key]


def kernel(**inputs):
    in_maps, nkeys = pack_inputs(
        inputs["value"], inputs["key"], inputs["query"], inputs["mask"],
        inputs["Wv"], inputs["Wk"], inputs["Wq"], inputs["Wo"], inputs["bo"])
    run = get_runner(nkeys=nkeys)
    res = run(in_maps)
    out = np.stack([res[c]["out"] for c in range(N_CORES)], axis=0)
    return out


# revision 8
# speedup vs baseline: 639.8290x; 8.8182x over previous
"""Trainium2 Bass kernel for single-head attention (B=8, S=2048, E=768).

Data-parallel over batch: core c computes batch c entirely.

Host-side packing (weight fusion + layout marshalling):
  Wkq   = Wk.T @ Wq / E      (bf16)  -- q/k projections fused into scores,
                                        with the 1/E softmax scale folded in
  WvoT2 = (Wo @ Wv).T        (bf16)  -- v/out projections fused, applied to
                                        value BEFORE attention (nkeys < S)
  query/key/value transposed+cast bf16 -> [E, *]; key/value gathered to the
  unmasked set (padded with masked keys, which contribute exp(-200)=0).
  maskb = 0 / -200 bias per key (fp32), precomputed on host.

Per-core device dataflow (PE contraction dim = partition dim):
  Hk[e',j]  = sum_e Wkq[e,e'] keyT[e,j]        (bf16)
  xv'[j,o]  = sum_e valueT[e,j] WvoT2[e,o] + bo[o]   (bf16; col 768 = 1.0)
  sT[j,i]   = sum_e' Hk[e',j] queryT[e',i]     = scaled q.k scores
  aT[j,i]   = exp(sT + maskb[j])               (ACT, bf16)
  U[i,o+]   = sum_j aT[j,i] xv'[j,o+]          (o+ includes ones col -> den[i])
  y[i,o]    = U[i,o] / U[i,768]                (recip + broadcast mult)
Since sum_j a[j,i]*(xv+bo)[j,o] = U[i,o] + den[i]*bo[o], normalizing by den
adds bo exactly. Output leaves in natural [S, E] orientation.
"""

import numpy as np

S, E, P = 2048, 768, 128
NE = E // P              # 6
IC = 512                 # score i-chunk
NIC = S // IC            # 4
N_CORES = 8
NKC = 1152               # compacted key count (9 j-tiles)
EP1 = E + 1              # 769: value' cols + ones column
EPAD = 772               # padded row length for xv' tile

_CACHE = {}


def _chunks(total, step=512):
    out = []
    o = 0
    while o < total:
        out.append((o, min(step, total - o)))
        o += step
    return out


def build_nc(n_iters=1, nkeys=NKC):
    import concourse.bacc as bacc
    import concourse.bass as bass
    import concourse.mybir as mybir
    import concourse.tile as tile

    F32 = mybir.dt.float32
    BF16 = mybir.dt.bfloat16
    F8 = mybir.dt.float8e4
    DR = mybir.MatmulPerfMode.DoubleRow
    AF = mybir.ActivationFunctionType
    ALU = mybir.AluOpType

    KJ = nkeys // P
    nc = bacc.Bacc("TRN2", target_bir_lowering=False, debug=False,
                   num_devices=N_CORES)

    xq_d = nc.dram_tensor("queryT", [E, S], F8, kind="ExternalInput").ap()
    xk_d = nc.dram_tensor("keyT", [E, nkeys], F8, kind="ExternalInput").ap()
    xv_d = nc.dram_tensor("valueT", [E, nkeys], BF16, kind="ExternalInput").ap()
    mb_d = nc.dram_tensor("maskb", [nkeys], F32, kind="ExternalInput").ap()
    wkq_d = nc.dram_tensor("Wkq", [E, E], F8, kind="ExternalInput").ap()
    wvot_d = nc.dram_tensor("WvoT2", [E, E], BF16, kind="ExternalInput").ap()
    bo_d = nc.dram_tensor("bo", [E], F32, kind="ExternalInput").ap()
    y_d = nc.dram_tensor("out", [S, E], F32, kind="ExternalOutput").ap()

    with tile.TileContext(nc) as tc, \
         tc.tile_pool(name="persist", bufs=1) as persist, \
         tc.tile_pool(name="ld2", bufs=2) as ld2, \
         tc.tile_pool(name="ld1", bufs=1) as ld1, \
         tc.tile_pool(name="rc", bufs=2) as recip_pool, \
         tc.tile_pool(name="ys", bufs=3) as y_pool, \
         tc.tile_pool(name="sp", bufs=2, space="PSUM") as psum_s, \
         tc.tile_pool(name="up", bufs=2, space="PSUM") as psum_u:

      for _it in range(n_iters):
        hk = persist.tile([P, NE, nkeys], F8, tag="hk")
        xvp = persist.tile([P, KJ, EPAD], BF16, tag="xvp")
        atile = persist.tile([P, KJ, IC], BF16, tag="at")

        # loads, in dependency order: phase A1 deps first, xqT last
        xkT = ld2.tile([P, NE, nkeys], F8, tag="xkt")
        nc.sync.dma_start(out=xkT, in_=xk_d.rearrange("(t p) j -> p t j", p=P))
        wkq = ld1.tile([P, NE, E], F8, tag="wt")
        nc.sync.dma_start(out=wkq, in_=wkq_d.rearrange("(t p) o -> p t o", p=P))
        xvT = ld2.tile([P, NE, nkeys], BF16, tag="xvt")
        nc.sync.dma_start(out=xvT, in_=xv_d.rearrange("(t p) j -> p t j", p=P))
        wvot = ld1.tile([P, NE, E], BF16, tag="wv")
        nc.sync.dma_start(out=wvot, in_=wvot_d.rearrange("(t p) o -> p t o", p=P))
        maskb = ld1.tile([P, KJ], F32, tag="mb")
        nc.sync.dma_start(out=maskb, in_=mb_d.rearrange("(t p) -> p t", p=P))
        bo_rep = ld1.tile([P, E], F32, tag="bo")
        bo_bc = bass.AP(tensor=bo_d.tensor, offset=bo_d.offset,
                        ap=[[0, P]] + list(bo_d.ap))
        nc.sync.dma_start(out=bo_rep, in_=bo_bc)
        xqT = ld2.tile([P, NE, S], F8, tag="xqt")
        nc.sync.dma_start(out=xqT, in_=xq_d.rearrange("(t p) i -> p t i", p=P))

        nc.vector.memset(xvp[:, :, E:E + 1], 1.0)

        # ------------- phase A1: Hk = Wkq.T @ keyT -------------
        for ept in range(NE):   # e' tile of Hk rows
            for o0, on in _chunks(nkeys):
                ps = psum_s.tile([P, on], F32, tag="s",
                                 name=f"hk{_it}_{ept}_{o0}")
                for et in range(NE // 2):
                    nc.tensor.matmul(
                        ps,
                        lhsT=wkq[:, 2 * et:2 * et + 2, ept * P:(ept + 1) * P],
                        rhs=xkT[:, 2 * et:2 * et + 2, o0:o0 + on],
                        start=(et == 0), stop=(et == NE // 2 - 1),
                        perf_mode=DR)
                nc.vector.tensor_copy(out=hk[:, ept, o0:o0 + on], in_=ps)

        # ------------- phase A2: xv' = valueT.T @ WvoT2 + bo -------------
        for jt in range(KJ):
            ps = psum_u.tile([P, E], F32, tag="u", name=f"xv{_it}_{jt}")
            for o0, on in _chunks(E):
                for et in range(NE):
                    nc.tensor.matmul(
                        ps[:, o0:o0 + on],
                        lhsT=xvT[:, et, jt * P:(jt + 1) * P],
                        rhs=wvot[:, et, o0:o0 + on],
                        start=(et == 0), stop=(et == NE - 1))
            nc.vector.tensor_tensor(
                out=xvp[:, jt, 0:E], in0=ps, in1=bo_rep, op=ALU.add)

        # ---------------- phase B: attention + output ----------------
        for ic in range(NIC):
            isl = slice(ic * IC, (ic + 1) * IC)
            for jt in range(KJ):
                s_ps = psum_s.tile([P, IC], F32, tag="s",
                                   name=f"sp{_it}_{ic}_{jt}")
                for ept in range(NE // 2):
                    nc.tensor.matmul(
                        s_ps,
                        lhsT=hk[:, 2 * ept:2 * ept + 2, jt * P:(jt + 1) * P],
                        rhs=xqT[:, 2 * ept:2 * ept + 2, isl],
                        start=(ept == 0), stop=(ept == NE // 2 - 1),
                        perf_mode=DR)
                nc.scalar.activation(
                    out=atile[:, jt, :], in_=s_ps, func=AF.Exp,
                    bias=maskb[:, jt:jt + 1], scale=1.0 / 1024.0)
            for it in range(IC // P):
                u_ps = psum_u.tile([P, EPAD], F32, tag="u",
                                   name=f"up{_it}_{ic}_{it}")
                for o0, on in ((0, 512), (512, EP1 - 512)):
                    for jt in range(KJ):
                        nc.tensor.matmul(
                            u_ps[:, o0:o0 + on],
                            lhsT=atile[:, jt, it * P:(it + 1) * P],
                            rhs=xvp[:, jt, o0:o0 + on],
                            start=(jt == 0), stop=(jt == KJ - 1))
                recip = recip_pool.tile([P, 1], F32, tag="rc")
                nc.vector.reciprocal(recip, u_ps[:, E:EP1])
                recip_bc = bass.AP(tensor=recip.tensor, offset=recip.offset,
                                   ap=[recip.ap[0], [0, E]])
                ysb = y_pool.tile([P, E], F32, tag="y")
                nc.vector.tensor_tensor(
                    out=ysb, in0=u_ps[:, 0:E], in1=recip_bc, op=ALU.mult)
                r0 = ic * IC + it * P
                nc.sync.dma_start(out=y_d[r0:r0 + P, :], in_=ysb)

    nc.compile()
    return nc


def get_nc(n_iters=1, nkeys=NKC):
    key = ("nc", n_iters, nkeys)
    if key not in _CACHE:
        _CACHE[key] = build_nc(n_iters, nkeys)
    return _CACHE[key]


def pack_inputs(value, key, query, mask, Wv, Wk, Wq, Wo, bo):
    """Host-side packing: per-core input maps (weight fusion + layouts)."""
    import ml_dtypes

    value = np.asarray(value, dtype=np.float32)
    key = np.asarray(key, dtype=np.float32)
    query = np.asarray(query, dtype=np.float32)
    mask = np.asarray(mask, dtype=np.int32)
    Wv = np.asarray(Wv, dtype=np.float32)
    Wk = np.asarray(Wk, dtype=np.float32)
    Wq = np.asarray(Wq, dtype=np.float32)
    Wo = np.asarray(Wo, dtype=np.float32)
    bo = np.asarray(bo, dtype=np.float32)

    wkq = np.ascontiguousarray(
        Wk.T @ Wq * (1024.0 / float(E))).astype(ml_dtypes.float8_e4m3)
    wvot = np.ascontiguousarray((Wo @ Wv).T).astype(ml_dtypes.bfloat16)

    # key compaction: keep unmasked keys, pad with masked ones (exp -> 0)
    idxs = []
    nkeys = NKC
    for c in range(N_CORES):
        m = mask[c, 0]
        keep = np.flatnonzero(m != 0)
        drop = np.flatnonzero(m == 0)
        if len(keep) > NKC or len(drop) == 0:
            nkeys = S
            break
        pad = np.full(NKC - len(keep), drop[0], dtype=np.int64)
        idxs.append(np.concatenate([keep, pad]))

    in_maps = []
    for c in range(N_CORES):
        if nkeys == S:
            kc, vc, mc = key[c], value[c], mask[c, 0]
        else:
            ix = idxs[c]
            kc, vc, mc = key[c][ix], value[c][ix], mask[c, 0][ix]
        maskb = np.where(mc != 0, 0.0, -200.0).astype(np.float32)
        in_maps.append({
            "queryT": np.ascontiguousarray(
                query[c].T).astype(ml_dtypes.float8_e4m3),
            "keyT": np.ascontiguousarray(kc.T).astype(ml_dtypes.float8_e4m3),
            "valueT": np.ascontiguousarray(vc.T).astype(ml_dtypes.bfloat16),
            "maskb": np.ascontiguousarray(maskb),
            "Wkq": wkq, "WvoT2": wvot,
            "bo": bo,
        })
    return in_maps, nkeys


def _make_runner(nc, n_cores):
    """Build a CACHED jitted executable for `nc` (sharded over n_cores).

    run_bass_kernel_spmd re-jits a fresh closure per call, so every call
    re-traces + re-serializes the NEFF. Building the jit once and reusing
    it makes repeat calls pay only dispatch + transfers + execution.
    """
    import jax
    from jax.sharding import Mesh, PartitionSpec
    from jax.experimental.shard_map import shard_map

    import concourse.mybir as mybir
    from concourse.bass2jax import (
        _bass_exec_p, install_neuronx_cc_hook, partition_id_tensor)

    install_neuronx_cc_hook()
    partition_name = (nc.partition_id_tensor.name
                      if nc.partition_id_tensor else None)
    in_names, out_names, out_avals, zero_outs = [], [], [], []
    for alloc in nc.m.functions[0].allocations:
        if not isinstance(alloc, mybir.MemoryLocationSet):
            continue
        name = alloc.memorylocations[0].name
        if alloc.kind == "ExternalInput":
            if name != partition_name:
                in_names.append(name)
        elif alloc.kind == "ExternalOutput":
            out_names.append(name)
            shape = tuple(alloc.tensor_shape)
            dtype = mybir.dt.np(alloc.dtype)
            out_avals.append(jax.core.ShapedArray(shape, dtype))
            zero_outs.append(np.zeros(shape, dtype))
    n_params = len(in_names)
    all_in_names = list(in_names) + list(out_names)
    if partition_name is not None:
        all_in_names.append(partition_name)

    def _body(*args):
        operands = list(args)
        if partition_name is not None:
            operands.append(partition_id_tensor())
        outs = _bass_exec_p.bind(
            *operands,
            out_avals=tuple(out_avals),
            in_names=tuple(all_in_names),
            out_names=tuple(out_names),
            lowering_input_output_aliases=(),
            sim_require_finite=True,
            sim_require_nnan=True,
            nc=nc,
        )
        return tuple(outs)

    devices = jax.devices()[:n_cores]
    mesh = Mesh(np.asarray(devices), ("core",))
    in_specs = (PartitionSpec("core"),) * (n_params + len(out_names))
    out_specs = (PartitionSpec("core"),) * len(out_names)
    fn = jax.jit(shard_map(_body, mesh=mesh, in_specs=in_specs,
                           out_specs=out_specs, check_rep=False))
    concat_zeros = [np.zeros((n_cores * z.shape[0], *z.shape[1:]), z.dtype)
                    for z in zero_outs]
    sharding = jax.sharding.NamedSharding(mesh, PartitionSpec("core"))

    def stage(in_maps):
        """device_put the packed inputs once; returns device arg list."""
        per_core = [[np.asarray(m[name]) for name in in_names]
                    for m in in_maps]
        concat_in = [
            np.concatenate([per_core[c][i] for c in range(n_cores)], axis=0)
            for i in range(n_params)]
        return [jax.device_put(a, sharding)
                for a in concat_in + concat_zeros]

    def run_dev(dev_args):
        """Execute on pre-staged device inputs; blocks until done."""
        out_arrs = fn(*dev_args)
        jax.block_until_ready(out_arrs)
        return out_arrs

    def dispatch(dev_args):
        """Execute without blocking (async); caller syncs."""
        return fn(*dev_args)

    def run(in_maps):
        out_arrs = run_dev(stage(in_maps))
        return [
            {name: np.asarray(out_arrs[i]).reshape(
                n_cores, *out_avals[i].shape)[c]
             for i, name in enumerate(out_names)}
            for c in range(n_cores)]

    run.stage = stage
    run.run_dev = run_dev
    run.dispatch = dispatch
    return run


def get_runner(n_iters=1, nkeys=NKC):
    key = ("runner", n_iters, nkeys)
    if key not in _CACHE:
        _CACHE[key] = _make_runner(get_nc(n_iters, nkeys), N_CORES)
    return _CACHE[key]


def kernel(**inputs):
    in_maps, nkeys = pack_inputs(
        inputs["value"], inputs["key"], inputs["query"], inputs["mask"],
        inputs["Wv"], inputs["Wk"], inputs["Wq"], inputs["Wo"], inputs["bo"])
    run = get_runner(nkeys=nkeys)
    res = run(in_maps)
    out = np.stack([res[c]["out"] for c in range(N_CORES)], axis=0)
    return out


# revision 9
# speedup vs baseline: 673.7835x; 1.0531x over previous
"""Trainium2 Bass kernel for single-head attention (B=8, S=2048, E=768).

Data-parallel over batch: core c computes batch c entirely.

Host-side packing (weight fusion + layout marshalling):
  Wkq   = Wk.T @ Wq * 1024/E (fp8e4m3) -- q/k projections fused into scores;
                                          1/E softmax scale and a x1024 fp8
                                          range scale folded in (undone by
                                          the exp's scale=1/1024)
  WvoT2 = (Wo @ Wv).T        (bf16)    -- v/out projections fused, applied to
                                          value BEFORE attention (nkeys < S)
  query/key transposed+cast fp8e4m3 -> [E, *]; value bf16; key/value gathered
  to the unmasked set (padded with masked keys, which contribute exp(-200)=0).
  maskb = 0 / -200 bias per key (fp32), precomputed on host.

Per-core device dataflow (PE contraction dim = partition dim):
  Hk[e',j]  = sum_e Wkq[e,e'] keyT[e,j]        (fp8 DoubleRow, stored fp8)
  xv'[j,o]  = sum_e valueT[e,j] WvoT2[e,o] + bo[o]   (bf16; col 768 = 1.0)
  sT[j,i]   = sum_e' Hk[e',j] queryT[e',i]     (fp8 DoubleRow, 256-contraction)
  aT[j,i]   = exp(sT/1024 + maskb[j])          (ACT, bf16)
  U[i,o+]   = sum_j aT[j,i] xv'[j,o+]          (o+ includes ones col -> den[i])
  y[i,o]    = U[i,o] / U[i,768]                (recip + broadcast mult)
Since sum_j a[j,i]*(xv+bo)[j,o] = U[i,o] + den[i]*bo[o], normalizing by den
adds bo exactly. Output leaves in natural [S, E] orientation.
"""

import numpy as np

S, E, P = 2048, 768, 128
NE = E // P              # 6
IC = 512                 # score i-chunk
NIC = S // IC            # 4
N_CORES = 8
NKC = 1152               # compacted key count (9 j-tiles)
EP1 = E + 1              # 769: value' cols + ones column
EPAD = 772               # padded row length for xv' tile

_CACHE = {}


def _chunks(total, step=512):
    out = []
    o = 0
    while o < total:
        out.append((o, min(step, total - o)))
        o += step
    return out


def build_nc(n_iters=1, nkeys=NKC):
    import concourse.bacc as bacc
    import concourse.bass as bass
    import concourse.mybir as mybir
    import concourse.tile as tile

    F32 = mybir.dt.float32
    BF16 = mybir.dt.bfloat16
    F8 = mybir.dt.float8e4
    DR = mybir.MatmulPerfMode.DoubleRow
    AF = mybir.ActivationFunctionType
    ALU = mybir.AluOpType

    KJ = nkeys // P
    nc = bacc.Bacc("TRN2", target_bir_lowering=False, debug=False,
                   num_devices=N_CORES)

    xq_d = nc.dram_tensor("queryT", [E, S], F8, kind="ExternalInput").ap()
    xk_d = nc.dram_tensor("keyT", [E, nkeys], F8, kind="ExternalInput").ap()
    xv_d = nc.dram_tensor("valueT", [E, nkeys], BF16, kind="ExternalInput").ap()
    mb_d = nc.dram_tensor("maskb", [nkeys], F32, kind="ExternalInput").ap()
    wkq_d = nc.dram_tensor("Wkq", [E, E], F8, kind="ExternalInput").ap()
    wvot_d = nc.dram_tensor("WvoT2", [E, E], BF16, kind="ExternalInput").ap()
    bo_d = nc.dram_tensor("bo", [E], F32, kind="ExternalInput").ap()
    y_d = nc.dram_tensor("out", [S, E], F32, kind="ExternalOutput").ap()

    with tile.TileContext(nc) as tc, \
         tc.tile_pool(name="persist", bufs=1) as persist, \
         tc.tile_pool(name="ld2", bufs=2) as ld2, \
         tc.tile_pool(name="ld1", bufs=1) as ld1, \
         tc.tile_pool(name="rc", bufs=2) as recip_pool, \
         tc.tile_pool(name="ys", bufs=3) as y_pool, \
         tc.tile_pool(name="sp", bufs=2, space="PSUM") as psum_s, \
         tc.tile_pool(name="up", bufs=2, space="PSUM") as psum_u:

      for _it in range(n_iters):
        hk = persist.tile([P, NE, nkeys], F8, tag="hk")
        xvp = persist.tile([P, KJ, EPAD], BF16, tag="xvp")
        atile = persist.tile([P, KJ, IC], BF16, tag="at")

        # loads, in dependency order: phase A1 deps first, xqT last
        xkT = ld2.tile([P, NE, nkeys], F8, tag="xkt")
        nc.sync.dma_start(out=xkT, in_=xk_d.rearrange("(t p) j -> p t j", p=P))
        wkq = ld1.tile([P, NE, E], F8, tag="wt")
        nc.sync.dma_start(out=wkq, in_=wkq_d.rearrange("(t p) o -> p t o", p=P))
        xvT = ld2.tile([P, NE, nkeys], BF16, tag="xvt")
        nc.sync.dma_start(out=xvT, in_=xv_d.rearrange("(t p) j -> p t j", p=P))
        wvot = ld1.tile([P, NE, E], BF16, tag="wv")
        nc.sync.dma_start(out=wvot, in_=wvot_d.rearrange("(t p) o -> p t o", p=P))
        maskb = ld1.tile([P, KJ], F32, tag="mb")
        nc.sync.dma_start(out=maskb, in_=mb_d.rearrange("(t p) -> p t", p=P))
        bo_rep = ld1.tile([P, E], F32, tag="bo")
        bo_bc = bass.AP(tensor=bo_d.tensor, offset=bo_d.offset,
                        ap=[[0, P]] + list(bo_d.ap))
        nc.sync.dma_start(out=bo_rep, in_=bo_bc)
        xqT = ld2.tile([P, NE, S], F8, tag="xqt")
        nc.sync.dma_start(out=xqT, in_=xq_d.rearrange("(t p) i -> p t i", p=P))

        nc.vector.memset(xvp[:, :, E:E + 1], 1.0)

        # ------------- phase A1: Hk = Wkq.T @ keyT -------------
        for ept in range(NE):   # e' tile of Hk rows
            for o0, on in _chunks(nkeys):
                ps = psum_s.tile([P, on], F32, tag="s",
                                 name=f"hk{_it}_{ept}_{o0}")
                for et in range(NE // 2):
                    nc.tensor.matmul(
                        ps,
                        lhsT=wkq[:, 2 * et:2 * et + 2, ept * P:(ept + 1) * P],
                        rhs=xkT[:, 2 * et:2 * et + 2, o0:o0 + on],
                        start=(et == 0), stop=(et == NE // 2 - 1),
                        perf_mode=DR)
                nc.vector.tensor_copy(out=hk[:, ept, o0:o0 + on], in_=ps)

        # ------------- phase A2: xv' = valueT.T @ WvoT2 + bo -------------
        for jt in range(KJ):
            ps = psum_u.tile([P, E], F32, tag="u", name=f"xv{_it}_{jt}")
            for o0, on in _chunks(E):
                for et in range(NE):
                    nc.tensor.matmul(
                        ps[:, o0:o0 + on],
                        lhsT=xvT[:, et, jt * P:(jt + 1) * P],
                        rhs=wvot[:, et, o0:o0 + on],
                        start=(et == 0), stop=(et == NE - 1))
            nc.vector.tensor_tensor(
                out=xvp[:, jt, 0:E], in0=ps, in1=bo_rep, op=ALU.add)

        # ---------------- phase B: attention + output ----------------
        for ic in range(NIC):
            isl = slice(ic * IC, (ic + 1) * IC)
            for jt in range(KJ):
                s_ps = psum_s.tile([P, IC], F32, tag="s",
                                   name=f"sp{_it}_{ic}_{jt}")
                for ept in range(NE // 2):
                    nc.tensor.matmul(
                        s_ps,
                        lhsT=hk[:, 2 * ept:2 * ept + 2, jt * P:(jt + 1) * P],
                        rhs=xqT[:, 2 * ept:2 * ept + 2, isl],
                        start=(ept == 0), stop=(ept == NE // 2 - 1),
                        perf_mode=DR)
                nc.scalar.activation(
                    out=atile[:, jt, :], in_=s_ps, func=AF.Exp,
                    bias=maskb[:, jt:jt + 1], scale=1.0 / 1024.0)
            for it in range(IC // P):
                u_ps = psum_u.tile([P, EPAD], F32, tag="u",
                                   name=f"up{_it}_{ic}_{it}")
                for o0, on in ((0, 512), (512, EP1 - 512)):
                    for jt in range(KJ):
                        nc.tensor.matmul(
                            u_ps[:, o0:o0 + on],
                            lhsT=atile[:, jt, it * P:(it + 1) * P],
                            rhs=xvp[:, jt, o0:o0 + on],
                            start=(jt == 0), stop=(jt == KJ - 1))
                recip = recip_pool.tile([P, 1], F32, tag="rc")
                nc.vector.reciprocal(recip, u_ps[:, E:EP1])
                recip_bc = bass.AP(tensor=recip.tensor, offset=recip.offset,
                                   ap=[recip.ap[0], [0, E]])
                ysb = y_pool.tile([P, E], F32, tag="y")
                nc.vector.tensor_tensor(
                    out=ysb, in0=u_ps[:, 0:E], in1=recip_bc, op=ALU.mult)
                r0 = ic * IC + it * P
                nc.sync.dma_start(out=y_d[r0:r0 + P, :], in_=ysb)

    nc.compile()
    return nc


def get_nc(n_iters=1, nkeys=NKC):
    key = ("nc", n_iters, nkeys)
    if key not in _CACHE:
        _CACHE[key] = build_nc(n_iters, nkeys)
    return _CACHE[key]


def pack_inputs(value, key, query, mask, Wv, Wk, Wq, Wo, bo):
    """Host-side packing: per-core input maps (weight fusion + layouts)."""
    import ml_dtypes

    value = np.asarray(value, dtype=np.float32)
    key = np.asarray(key, dtype=np.float32)
    query = np.asarray(query, dtype=np.float32)
    mask = np.asarray(mask, dtype=np.int32)
    Wv = np.asarray(Wv, dtype=np.float32)
    Wk = np.asarray(Wk, dtype=np.float32)
    Wq = np.asarray(Wq, dtype=np.float32)
    Wo = np.asarray(Wo, dtype=np.float32)
    bo = np.asarray(bo, dtype=np.float32)

    wkq = np.ascontiguousarray(
        Wk.T @ Wq * (1024.0 / float(E))).astype(ml_dtypes.float8_e4m3)
    wvot = np.ascontiguousarray((Wo @ Wv).T).astype(ml_dtypes.bfloat16)

    # key compaction: keep unmasked keys, pad with masked ones (exp -> 0)
    idxs = []
    nkeys = NKC
    for c in range(N_CORES):
        m = mask[c, 0]
        keep = np.flatnonzero(m != 0)
        drop = np.flatnonzero(m == 0)
        if len(keep) > NKC or len(drop) == 0:
            nkeys = S
            break
        pad = np.full(NKC - len(keep), drop[0], dtype=np.int64)
        idxs.append(np.concatenate([keep, pad]))

    in_maps = []
    for c in range(N_CORES):
        if nkeys == S:
            kc, vc, mc = key[c], value[c], mask[c, 0]
        else:
            ix = idxs[c]
            kc, vc, mc = key[c][ix], value[c][ix], mask[c, 0][ix]
        maskb = np.where(mc != 0, 0.0, -200.0).astype(np.float32)
        in_maps.append({
            "queryT": np.ascontiguousarray(
                query[c].T).astype(ml_dtypes.float8_e4m3),
            "keyT": np.ascontiguousarray(kc.T).astype(ml_dtypes.float8_e4m3),
            "valueT": np.ascontiguousarray(vc.T).astype(ml_dtypes.bfloat16),
            "maskb": np.ascontiguousarray(maskb),
            "Wkq": wkq, "WvoT2": wvot,
            "bo": bo,
        })
    return in_maps, nkeys


def _make_runner(nc, n_cores):
    """Build a CACHED jitted executable for `nc` (sharded over n_cores).

    run_bass_kernel_spmd re-jits a fresh closure per call, so every call
    re-traces + re-serializes the NEFF. Building the jit once and reusing
    it makes repeat calls pay only dispatch + transfers + execution.
    """
    import jax
    from jax.sharding import Mesh, PartitionSpec
    from jax.experimental.shard_map import shard_map

    import concourse.mybir as mybir
    from concourse.bass2jax import (
        _bass_exec_p, install_neuronx_cc_hook, partition_id_tensor)

    install_neuronx_cc_hook()
    partition_name = (nc.partition_id_tensor.name
                      if nc.partition_id_tensor else None)
    in_names, out_names, out_avals, zero_outs = [], [], [], []
    for alloc in nc.m.functions[0].allocations:
        if not isinstance(alloc, mybir.MemoryLocationSet):
            continue
        name = alloc.memorylocations[0].name
        if alloc.kind == "ExternalInput":
            if name != partition_name:
                in_names.append(name)
        elif alloc.kind == "ExternalOutput":
            out_names.append(name)
            shape = tuple(alloc.tensor_shape)
            dtype = mybir.dt.np(alloc.dtype)
            out_avals.append(jax.core.ShapedArray(shape, dtype))
            zero_outs.append(np.zeros(shape, dtype))
    n_params = len(in_names)
    all_in_names = list(in_names) + list(out_names)
    if partition_name is not None:
        all_in_names.append(partition_name)

    def _body(*args):
        operands = list(args)
        if partition_name is not None:
            operands.append(partition_id_tensor())
        outs = _bass_exec_p.bind(
            *operands,
            out_avals=tuple(out_avals),
            in_names=tuple(all_in_names),
            out_names=tuple(out_names),
            lowering_input_output_aliases=(),
            sim_require_finite=True,
            sim_require_nnan=True,
            nc=nc,
        )
        return tuple(outs)

    devices = jax.devices()[:n_cores]
    mesh = Mesh(np.asarray(devices), ("core",))
    in_specs = (PartitionSpec("core"),) * (n_params + len(out_names))
    out_specs = (PartitionSpec("core"),) * len(out_names)
    fn = jax.jit(shard_map(_body, mesh=mesh, in_specs=in_specs,
                           out_specs=out_specs, check_rep=False))
    concat_zeros = [np.zeros((n_cores * z.shape[0], *z.shape[1:]), z.dtype)
                    for z in zero_outs]
    sharding = jax.sharding.NamedSharding(mesh, PartitionSpec("core"))

    def stage(in_maps):
        """device_put the packed inputs once; returns device arg list."""
        per_core = [[np.asarray(m[name]) for name in in_names]
                    for m in in_maps]
        concat_in = [
            np.concatenate([per_core[c][i] for c in range(n_cores)], axis=0)
            for i in range(n_params)]
        return [jax.device_put(a, sharding)
                for a in concat_in + concat_zeros]

    def run_dev(dev_args):
        """Execute on pre-staged device inputs; blocks until done."""
        out_arrs = fn(*dev_args)
        jax.block_until_ready(out_arrs)
        return out_arrs

    def dispatch(dev_args):
        """Execute without blocking (async); caller syncs."""
        return fn(*dev_args)

    def run(in_maps):
        out_arrs = run_dev(stage(in_maps))
        return [
            {name: np.asarray(out_arrs[i]).reshape(
                n_cores, *out_avals[i].shape)[c]
             for i, name in enumerate(out_names)}
            for c in range(n_cores)]

    run.stage = stage
    run.run_dev = run_dev
    run.dispatch = dispatch
    return run


def get_runner(n_iters=1, nkeys=NKC):
    key = ("runner", n_iters, nkeys)
    if key not in _CACHE:
        _CACHE[key] = _make_runner(get_nc(n_iters, nkeys), N_CORES)
    return _CACHE[key]


def kernel(**inputs):
    in_maps, nkeys = pack_inputs(
        inputs["value"], inputs["key"], inputs["query"], inputs["mask"],
        inputs["Wv"], inputs["Wk"], inputs["Wq"], inputs["Wo"], inputs["bo"])
    run = get_runner(nkeys=nkeys)
    res = run(in_maps)
    out = np.stack([res[c]["out"] for c in range(N_CORES)], axis=0)
    return out


# revision 14
# speedup vs baseline: 985.6072x; 1.4628x over previous
"""Trainium2 Bass kernel for single-head attention (B=8, S=2048, E=768).

Data-parallel over batch: core c computes batch c entirely.

Host-side packing (weight fusion + layout marshalling):
  Wkq   = Wk.T @ Wq * 1024/E (fp8e4m3) -- q/k projections fused into scores;
                                          1/E softmax scale and a x1024 fp8
                                          range scale folded in (undone by
                                          the exp's scale=1/1024)
  WvoT2 = (Wo @ Wv).T        (bf16)    -- v/out projections fused, applied to
                                          value BEFORE attention (nkeys < S)
  query/key transposed+cast fp8e4m3 -> [E, *]; value bf16; key/value gathered
  to the unmasked set (padded with masked keys, which contribute exp(-200)=0).
  maskb = 0 / -200 bias per key (fp32), precomputed on host.

Per-core device dataflow (PE contraction dim = partition dim):
  Hk[e',j]  = sum_e Wkq[e,e'] keyT[e,j]        (fp8 DoubleRow, stored fp8)
  xv'[j,o]  = sum_e valueT[e,j] WvoT2[e,o] + bo[o]   (bf16; col 768 = 1.0)
  sT[j,i]   = sum_e' Hk[e',j] queryT[e',i]     (fp8 DoubleRow, 256-contraction)
  aT[j,i]   = exp(sT/1024 + maskb[j])          (ACT, bf16)
  U[i,o+]   = sum_j aT[j,i] xv'[j,o+]          (o+ includes ones col -> den[i])
  y[i,o]    = U[i,o] / U[i,768]                (recip + broadcast mult)
Since sum_j a[j,i]*(xv+bo)[j,o] = U[i,o] + den[i]*bo[o], normalizing by den
adds bo exactly. Output leaves in natural [S, E] orientation.
"""

import numpy as np

S, E, P = 2048, 768, 128
NE = E // P              # 6
IC = 512                 # score i-chunk
NIC = S // IC            # 4
N_CORES = 8
NKC = 1152               # compacted key count (9 j-tiles)
EP1 = E + 1              # 769: value' cols + ones column
EPAD = 772               # padded row length for xv' tile

_CACHE = {}


def _chunks(total, step=512):
    out = []
    o = 0
    while o < total:
        out.append((o, min(step, total - o)))
        o += step
    return out


def build_nc(n_iters=1, nkeys=NKC):
    import concourse.bacc as bacc
    import concourse.bass as bass
    import concourse.mybir as mybir
    import concourse.tile as tile

    F32 = mybir.dt.float32
    BF16 = mybir.dt.bfloat16
    F8 = mybir.dt.float8e4
    DR = mybir.MatmulPerfMode.DoubleRow
    AF = mybir.ActivationFunctionType
    ALU = mybir.AluOpType

    KJ = nkeys // P
    KJP = KJ + (KJ % 2)      # j-tiles padded to even for DoubleRow pairing
    nc = bacc.Bacc("TRN2", target_bir_lowering=False, debug=False,
                   num_devices=N_CORES)

    xq_d = nc.dram_tensor("queryT", [E, S], F8, kind="ExternalInput").ap()
    xk_d = nc.dram_tensor("keyT", [E, nkeys], F8, kind="ExternalInput").ap()
    xv_d = nc.dram_tensor("valueT", [E, nkeys], BF16, kind="ExternalInput").ap()
    mb_d = nc.dram_tensor("maskb", [nkeys], F32, kind="ExternalInput").ap()
    wkq_d = nc.dram_tensor("Wkq", [E, E], F8, kind="ExternalInput").ap()
    wvot_d = nc.dram_tensor("WvoT2", [E, E], BF16, kind="ExternalInput").ap()
    bo_d = nc.dram_tensor("bo", [E], F32, kind="ExternalInput").ap()
    cv_d = nc.dram_tensor("cvec", [EPAD], BF16, kind="ExternalInput").ap()
    mv_d = nc.dram_tensor("mvec", [nkeys], BF16, kind="ExternalInput").ap()
    y_d = nc.dram_tensor("out", [S, E], F32, kind="ExternalOutput").ap()

    with tile.TileContext(nc) as tc, \
         tc.tile_pool(name="persist", bufs=1) as persist, \
         tc.tile_pool(name="ld2", bufs=2) as ld2, \
         tc.tile_pool(name="ld1", bufs=1) as ld1, \
         tc.tile_pool(name="rc", bufs=2) as recip_pool, \
         tc.tile_pool(name="ab", bufs=2) as ab_pool, \
         tc.tile_pool(name="ys", bufs=3) as y_pool, \
         tc.tile_pool(name="sp", bufs=2, space="PSUM") as psum_s, \
         tc.tile_pool(name="up", bufs=2, space="PSUM") as psum_u:

      for _it in range(n_iters):
        hk = persist.tile([P, NE, nkeys], F8, tag="hk")
        xvp = persist.tile([P, KJP, EPAD], F8, tag="xvp")
        atile = persist.tile([P, KJP, IC], F8, tag="at")

        # loads, in dependency order: phase A1 deps first, xqT last
        xkT = ld2.tile([P, NE, nkeys], F8, tag="xkt")
        nc.sync.dma_start(out=xkT, in_=xk_d.rearrange("(t p) j -> p t j", p=P))
        wkq = ld1.tile([P, NE, E], F8, tag="wt")
        nc.sync.dma_start(out=wkq, in_=wkq_d.rearrange("(t p) o -> p t o", p=P))
        xvT = ld2.tile([P, NE, nkeys], BF16, tag="xvt")
        nc.sync.dma_start(out=xvT, in_=xv_d.rearrange("(t p) j -> p t j", p=P))
        wvot = ld1.tile([P, NE, E], BF16, tag="wv")
        nc.sync.dma_start(out=wvot, in_=wvot_d.rearrange("(t p) o -> p t o", p=P))
        maskb = ld1.tile([P, KJ], F32, tag="mb")
        nc.sync.dma_start(out=maskb, in_=mb_d.rearrange("(t p) -> p t", p=P))
        bo_rep = ld1.tile([P, E], F32, tag="bo")
        bo_bc = bass.AP(tensor=bo_d.tensor, offset=bo_d.offset,
                        ap=[[0, P]] + list(bo_d.ap))
        nc.sync.dma_start(out=bo_rep, in_=bo_bc)
        m_sb = ld1.tile([P, KJ], BF16, tag="mv")
        nc.sync.dma_start(out=m_sb, in_=mv_d.rearrange("(t p) -> p t", p=P))
        c_sb = ld1.tile([1, EPAD], BF16, tag="cv")
        c_bc = bass.AP(tensor=cv_d.tensor, offset=cv_d.offset,
                       ap=[[0, 1]] + list(cv_d.ap))
        nc.sync.dma_start(out=c_sb, in_=c_bc)
        ones_row = ld1.tile([1, P], BF16, tag="or")
        nc.vector.memset(ones_row, 1.0)
        xqT = ld2.tile([P, NE, S], F8, tag="xqt")
        nc.sync.dma_start(out=xqT, in_=xq_d.rearrange("(t p) i -> p t i", p=P))

        nc.vector.memset(xvp[:, :, E:E + 1], 1.0)
        if KJP > KJ:
            nc.vector.memset(xvp[:, KJ:KJP, :], 0.0)
            nc.vector.memset(atile[:, KJ:KJP, :], 0.0)

        # ------------- phase A1: Hk = Wkq.T @ keyT -------------
        for ept in range(NE):   # e' tile of Hk rows
            for o0, on in _chunks(nkeys):
                ps = psum_s.tile([P, on], F32, tag="s",
                                 name=f"hk{_it}_{ept}_{o0}")
                for et in range(NE // 2):
                    nc.tensor.matmul(
                        ps,
                        lhsT=wkq[:, 2 * et:2 * et + 2, ept * P:(ept + 1) * P],
                        rhs=xkT[:, 2 * et:2 * et + 2, o0:o0 + on],
                        start=(et == 0), stop=(et == NE // 2 - 1),
                        perf_mode=DR)
                nc.scalar.copy(out=hk[:, ept, o0:o0 + on], in_=ps)

        # ------------- phase A2: xv' = valueT.T @ WvoT2 + bo -------------
        for jt in range(KJ):
            ps = psum_u.tile([P, E], F32, tag="u", name=f"xv{_it}_{jt}")
            for o0, on in _chunks(E):
                for et in range(NE):
                    nc.tensor.matmul(
                        ps[:, o0:o0 + on],
                        lhsT=xvT[:, et, jt * P:(jt + 1) * P],
                        rhs=wvot[:, et, o0:o0 + on],
                        start=(et == 0), stop=(et == NE - 1))
            nc.vector.tensor_tensor(
                out=xvp[:, jt, 0:E], in0=ps, in1=bo_rep, op=ALU.add)

        # ---------------- phase B: attention + output ----------------
        for ic in range(NIC):
            isl = slice(ic * IC, (ic + 1) * IC)
            at_bf = ab_pool.tile([P, KJ, IC], F32, tag="ab")
            for jt in range(KJ):
                s_ps = psum_s.tile([P, IC], F32, tag="s",
                                   name=f"sp{_it}_{ic}_{jt}")
                for ept in range(NE // 2):
                    nc.tensor.matmul(
                        s_ps,
                        lhsT=hk[:, 2 * ept:2 * ept + 2, jt * P:(jt + 1) * P],
                        rhs=xqT[:, 2 * ept:2 * ept + 2, isl],
                        start=(ept == 0), stop=(ept == NE // 2 - 1),
                        perf_mode=DR)
                nc.scalar.activation(
                    out=at_bf[:, jt, :], in_=s_ps, func=AF.Exp,
                    bias=maskb[:, jt:jt + 1], scale=1.0 / 1024.0)
            # centered weights, two batched ops so the first overlaps
            # the remaining exps of this i-chunk
            half = KJ // 2
            for j0, j1 in ((0, half), (half, KJ)):
                esz = m_sb.dtype.size if hasattr(m_sb, "dtype") else 2
                msl = m_sb[:, j0:j1]
                m_bc3 = bass.AP(tensor=msl.tensor, offset=msl.offset,
                                ap=[msl.ap[0], msl.ap[1], [0, IC]])
                nc.vector.tensor_tensor(
                    out=atile[:, j0:j1, :], in0=at_bf[:, j0:j1, :],
                    in1=m_bc3, op=ALU.subtract)
            for it in range(IC // P):
                u_ps = psum_u.tile([P, EPAD], F32, tag="u",
                                   name=f"up{_it}_{ic}_{it}")
                for o0, on in ((0, 512), (512, EP1 - 512)):
                    for tp in range(KJP // 2):
                        nc.tensor.matmul(
                            u_ps[:, o0:o0 + on],
                            lhsT=atile[:, 2 * tp:2 * tp + 2,
                                       it * P:(it + 1) * P],
                            rhs=xvp[:, 2 * tp:2 * tp + 2, o0:o0 + on],
                            start=(tp == 0), stop=False,
                            perf_mode=DR)
                    nc.tensor.matmul(
                        u_ps[:, o0:o0 + on],
                        lhsT=ones_row,
                        rhs=c_sb[:, o0:o0 + on],
                        start=False, stop=True)
                recip = recip_pool.tile([P, 1], F32, tag="rc")
                nc.vector.reciprocal(recip, u_ps[:, E:EP1])
                recip_bc = bass.AP(tensor=recip.tensor, offset=recip.offset,
                                   ap=[recip.ap[0], [0, E]])
                ysb = y_pool.tile([P, E], F32, tag="y")
                nc.vector.tensor_tensor(
                    out=ysb, in0=u_ps[:, 0:E], in1=recip_bc, op=ALU.mult)
                r0 = ic * IC + it * P
                nc.sync.dma_start(out=y_d[r0:r0 + P, :], in_=ysb)

    nc.compile()
    return nc


def get_nc(n_iters=1, nkeys=NKC):
    key = ("nc", n_iters, nkeys)
    if key not in _CACHE:
        _CACHE[key] = build_nc(n_iters, nkeys)
    return _CACHE[key]


def pack_inputs(value, key, query, mask, Wv, Wk, Wq, Wo, bo):
    """Host-side packing: per-core input maps (weight fusion + layouts)."""
    import ml_dtypes

    value = np.asarray(value, dtype=np.float32)
    key = np.asarray(key, dtype=np.float32)
    query = np.asarray(query, dtype=np.float32)
    mask = np.asarray(mask, dtype=np.int32)
    Wv = np.asarray(Wv, dtype=np.float32)
    Wk = np.asarray(Wk, dtype=np.float32)
    Wq = np.asarray(Wq, dtype=np.float32)
    Wo = np.asarray(Wo, dtype=np.float32)
    bo = np.asarray(bo, dtype=np.float32)

    wkq = np.ascontiguousarray(
        Wk.T @ Wq * (1024.0 / float(E))).astype(ml_dtypes.float8_e4m3)
    wvot = np.ascontiguousarray((Wo @ Wv).T).astype(ml_dtypes.bfloat16)

    # key compaction: keep unmasked keys, pad with masked ones (exp -> 0)
    idxs = []
    nkeys = NKC
    for c in range(N_CORES):
        m = mask[c, 0]
        keep = np.flatnonzero(m != 0)
        drop = np.flatnonzero(m == 0)
        if len(keep) > NKC or len(drop) == 0:
            nkeys = S
            break
        pad = np.full(NKC - len(keep), drop[0], dtype=np.int64)
        idxs.append(np.concatenate([keep, pad]))

    in_maps = []
    for c in range(N_CORES):
        if nkeys == S:
            kc, vc, mc = key[c], value[c], mask[c, 0]
        else:
            ix = idxs[c]
            kc, vc, mc = key[c][ix], value[c][ix], mask[c, 0][ix]
        maskb = np.where(mc != 0, 0.0, -200.0).astype(np.float32)
        mvec = (mc != 0).astype(np.float32)
        # exact query-independent attention mean: c = sum_unmasked xv' rows
        kcount = float(mvec.sum())
        vsum = (vc.astype(np.float64) * (mc != 0)[:, None]).sum(axis=0)
        c_full = vsum @ (Wo @ Wv).T.astype(np.float64) + kcount * bo
        cvec = np.zeros(EPAD, np.float64)
        cvec[0:E] = c_full
        cvec[E] = kcount
        in_maps.append({
            "queryT": np.ascontiguousarray(
                query[c].T).astype(ml_dtypes.float8_e4m3),
            "keyT": np.ascontiguousarray(kc.T).astype(ml_dtypes.float8_e4m3),
            "valueT": np.ascontiguousarray(vc.T).astype(ml_dtypes.bfloat16),
            "maskb": np.ascontiguousarray(maskb),
            "Wkq": wkq, "WvoT2": wvot,
            "bo": bo,
            "cvec": cvec.astype(ml_dtypes.bfloat16),
            "mvec": mvec.astype(ml_dtypes.bfloat16),
        })
    return in_maps, nkeys


def _make_runner(nc, n_cores):
    """Build a CACHED jitted executable for `nc` (sharded over n_cores).

    run_bass_kernel_spmd re-jits a fresh closure per call, so every call
    re-traces + re-serializes the NEFF. Building the jit once and reusing
    it makes repeat calls pay only dispatch + transfers + execution.
    """
    import jax
    from jax.sharding import Mesh, PartitionSpec
    from jax.experimental.shard_map import shard_map

    import concourse.mybir as mybir
    from concourse.bass2jax import (
        _bass_exec_p, install_neuronx_cc_hook, partition_id_tensor)

    install_neuronx_cc_hook()
    partition_name = (nc.partition_id_tensor.name
                      if nc.partition_id_tensor else None)
    in_names, out_names, out_avals, zero_outs = [], [], [], []
    for alloc in nc.m.functions[0].allocations:
        if not isinstance(alloc, mybir.MemoryLocationSet):
            continue
        name = alloc.memorylocations[0].name
        if alloc.kind == "ExternalInput":
            if name != partition_name:
                in_names.append(name)
        elif alloc.kind == "ExternalOutput":
            out_names.append(name)
            shape = tuple(alloc.tensor_shape)
            dtype = mybir.dt.np(alloc.dtype)
            out_avals.append(jax.core.ShapedArray(shape, dtype))
            zero_outs.append(np.zeros(shape, dtype))
    n_params = len(in_names)
    all_in_names = list(in_names) + list(out_names)
    if partition_name is not None:
        all_in_names.append(partition_name)

    def _body(*args):
        operands = list(args)
        if partition_name is not None:
            operands.append(partition_id_tensor())
        outs = _bass_exec_p.bind(
            *operands,
            out_avals=tuple(out_avals),
            in_names=tuple(all_in_names),
            out_names=tuple(out_names),
            lowering_input_output_aliases=(),
            sim_require_finite=True,
            sim_require_nnan=True,
            nc=nc,
        )
        return tuple(outs)

    devices = jax.devices()[:n_cores]
    mesh = Mesh(np.asarray(devices), ("core",))
    in_specs = (PartitionSpec("core"),) * (n_params + len(out_names))
    out_specs = (PartitionSpec("core"),) * len(out_names)
    fn = jax.jit(shard_map(_body, mesh=mesh, in_specs=in_specs,
                           out_specs=out_specs, check_rep=False))
    concat_zeros = [np.zeros((n_cores * z.shape[0], *z.shape[1:]), z.dtype)
                    for z in zero_outs]
    sharding = jax.sharding.NamedSharding(mesh, PartitionSpec("core"))

    def stage(in_maps):
        """device_put the packed inputs once; returns device arg list."""
        per_core = [[np.asarray(m[name]) for name in in_names]
                    for m in in_maps]
        concat_in = [
            np.concatenate([per_core[c][i] for c in range(n_cores)], axis=0)
            for i in range(n_params)]
        return [jax.device_put(a, sharding)
                for a in concat_in + concat_zeros]

    def run_dev(dev_args):
        """Execute on pre-staged device inputs; blocks until done."""
        out_arrs = fn(*dev_args)
        jax.block_until_ready(out_arrs)
        return out_arrs

    def dispatch(dev_args):
        """Execute without blocking (async); caller syncs."""
        return fn(*dev_args)

    def run(in_maps):
        out_arrs = run_dev(stage(in_maps))
        return [
            {name: np.asarray(out_arrs[i]).reshape(
                n_cores, *out_avals[i].shape)[c]
             for i, name in enumerate(out_names)}
            for c in range(n_cores)]

    run.stage = stage
    run.run_dev = run_dev
    run.dispatch = dispatch
    return run


def get_runner(n_iters=1, nkeys=NKC):
    key = ("runner", n_iters, nkeys)
    if key not in _CACHE:
        _CACHE[key] = _make_runner(get_nc(n_iters, nkeys), N_CORES)
    return _CACHE[key]


def kernel(**inputs):
    in_maps, nkeys = pack_inputs(
        inputs["value"], inputs["key"], inputs["query"], inputs["mask"],
        inputs["Wv"], inputs["Wk"], inputs["Wq"], inputs["Wo"], inputs["bo"])
    run = get_runner(nkeys=nkeys)
    res = run(in_maps)
    out = np.stack([res[c]["out"] for c in range(N_CORES)], axis=0)
    return out
